# revision 15
# baseline (speedup 1.0000x reference)
"""Cluster-wise linear (MoE-style dense routing) Trainium2 kernel, v2.

Computes out[t,o] = sum_c prob[t,c] * (x[t] @ W[c].T + b[c])[o] for
x (128,321,336) f32, prob (128,321,8), W (8,96,336), b (8,96).

Strategy: data-parallel over 8 NeuronCores (tokens = batch*n_vars split
evenly, 5136/core padded to 41 tiles of 128).

v3 design (vs the v1 baseline's 2688 PE rows/tile):
  - x is transposed + cast to bf16 on the HOST (xT [384, 5248] with a
    ones row at 336 so the bias folds into the matmul). This removes the
    3 PE transposes + DVE copy per tile (384 of 2688 PE rows) and halves
    x HBM traffic. PE per tile = 6 matmuls = 2304 rows ~ 960ns @2.4GHz,
    the roofline engine (41 tiles -> ~39.4us/core steady state).
  - x loads batched 4 tiles per DMA (1024B contiguous lines, avoids the
    <512B descriptor half-rate penalty), issued from sync (HWDGE).
    Output stores batched 2 tiles per DMA on gpsimd (SWDGE).
  - output written bf16 (host casts back to f32): halves out traffic and
    keeps the DVE 2x packing mode on the cluster reduce.
  - stage 2 is PAIR-batched: PSUM = 2 mega-tiles of 2048 f32 (4 banks
    each), a pair of tiles lives at bank-aligned 1024-col slots; one
    Act copy + one DVE mult + one DVE reduce covers 2 tiles, halving
    per-instruction overheads and semaphore traffic.
Per-tile engine budget: PE 6 MM ~960ns (critical), Act copy ~710ns,
DVE mult+reduce ~860ns, DMA engines ~550ns, all hidden behind PE.
"""

import numpy as np
import ml_dtypes

import concourse.bass as bass
import concourse.mybir as mybir
import concourse.tile as tile
from concourse.bass_utils import run_bass_kernel_spmd

N_CORES = 8
BSZ, N_VARS, IN_DIM, OUT_DIM, N_CLUSTER = 128, 321, 336, 96, 8
TOK = BSZ * N_VARS            # 41088
TPC = TOK // N_CORES          # 5136 tokens per core
P = 128
N_TILES = (TPC + P - 1) // P  # 41
TPAD = N_TILES * P            # 5248 padded tokens per core
N_PAIR = (N_TILES + 1) // 2   # 21 output pairs
OPAD = N_PAIR * 2 * P         # 5376 padded rows in the out tensor
IN_P = 384                    # padded input dim: 336 data + 1 ones + 47 zeros
CO = OUT_DIM * N_CLUSTER      # 768, o-major: co = o*8 + c
XBLK = 4                      # tiles per x-load DMA
N_XBLK = N_TILES // XBLK      # 10 full blocks
XREM = N_TILES - N_XBLK * XBLK  # 1 leftover tile


def split_multi_waits(nc):
    """This walrus build only supports one sync-wait per instruction; hoist
    extra waits onto same-engine nops inserted immediately before."""
    n_split = 0
    for fn in nc.m.functions:
        for bb in fn.blocks:
            insts = bb.instructions
            out = []
            changed = False
            for inst in insts:
                si = inst.sync_info
                if si is not None and si.on_wait and len(si.on_wait) > 1:
                    waits = list(si.on_wait)
                    del si.on_wait[1:]
                    si.on_wait[0] = waits[-1]
                    for w in waits[:-1]:
                        nop = mybir.InstNoOp(
                            name=f"{inst.name}-wsplit-{n_split}", ins=[], outs=[]
                        )
                        n_split += 1
                        nop.engine = inst.engine
                        nop.sync_info = mybir.SyncInfo(on_wait=[w], on_update=[])
                        out.append(nop)
                        changed = True
                out.append(inst)
            if changed:
                insts[:] = out
    return n_split


def build_nc(nrep: int = 1, bufs: int = 3, loop_iters: int | None = None,
             do_load=True, do_matmul=True, do_stage2=True, do_store=True,
             x_dma_sync=True, out_dma_sync=False):
    nc = bass.Bass()
    dt = mybir.dt
    x_d = nc.dram_tensor("xt", [IN_P, TPAD], dt.bfloat16, kind="ExternalInput")
    p_d = nc.dram_tensor(
        "probp", [P, N_TILES * N_CLUSTER], dt.bfloat16, kind="ExternalInput"
    )
    w_d = nc.dram_tensor("wt", [IN_P, CO], dt.bfloat16, kind="ExternalInput")
    o_d = nc.dram_tensor("out", [OPAD, OUT_DIM], dt.bfloat16, kind="ExternalOutput")

    x_eng = nc.sync if x_dma_sync else nc.gpsimd

    with tile.TileContext(nc) as tc:
        with (
            tc.tile_pool(name="const", bufs=1) as const,
            tc.tile_pool(name="work", bufs=1) as work,
            tc.tile_pool(name="psum", bufs=1, space="PSUM") as psum,
        ):
            # one-time loads
            wtb = const.tile([P, 3 * CO], dt.bfloat16)
            wtb3 = wtb.rearrange("p (k n) -> p k n", k=3)
            nc.gpsimd.dma_start(wtb3[:], w_d.rearrange("(k p) n -> p k n", p=P))
            pball = const.tile([P, N_TILES * N_CLUSTER], dt.bfloat16)
            nc.gpsimd.dma_start(pball[:], p_d[:])
            pb3 = pball.rearrange("p (j c) -> p j c", c=N_CLUSTER)

            # rings. PSUM: 2 mega-tiles of 2048 f32 (4 banks each = all 8
            # banks); each holds a PAIR of tiles at bank-aligned 1024-col
            # slots so stage-2 handles 2 tiles per instruction.
            xb_ring = [
                work.tile([P, 3 * XBLK * P], dt.bfloat16, name=f"xb{i}")
                for i in range(2)
            ]
            xb_v = [t.rearrange("p (k t) -> p k t", k=3) for t in xb_ring]
            y_ring = [
                psum.tile([P, 2048], dt.float32, name=f"yps{i}") for i in range(2)
            ]
            ysb_ring = [
                work.tile([P, 2 * CO], dt.bfloat16, name=f"ysb{i}") for i in range(2)
            ]
            ysb_v = [t.rearrange("p (h n) -> p h n", h=2) for t in ysb_ring]
            z_ring = [
                work.tile([P, 2 * CO], dt.bfloat16, name=f"z{i}") for i in range(2)
            ]
            z_v = [t.rearrange("p (h o c) -> p h o c", h=2, c=N_CLUSTER)
                   for t in z_ring]
            o_ring = [
                work.tile([P, 2 * OUT_DIM], dt.bfloat16, name=f"osb{i}")
                for i in range(2)
            ]
            o_v = [t.rearrange("p (h o) -> p h o", h=2) for t in o_ring]

            # DRAM views
            x_full = x_d.rearrange("(k p) t -> p k t", p=P)
            o_blk = o_d.rearrange("(q h p) o -> p q h o", h=2, p=P)

            # the lone final tile stores a half-garbage pair into padded
            # DRAM rows; memset once so the padding lane is finite
            for t in o_ring:
                nc.vector.memset(t[:], 0.0)

            def tile_body(j: int):
                if do_load:
                    if j % XBLK == 0 and j // XBLK < N_XBLK:
                        q = j // XBLK
                        xb = xb_v[q % 2]
                        x_eng.dma_start(
                            xb[:], x_full[:, :, q * XBLK * P : (q + 1) * XBLK * P]
                        )
                    elif j == N_XBLK * XBLK:
                        # leftover tile: load into first part of next ring slot
                        xb = xb_v[(j // XBLK) % 2]
                        x_eng.dma_start(
                            xb[:, :, 0:P], x_full[:, :, j * P : (j + 1) * P]
                        )
                xb = xb_v[(j // XBLK) % 2]
                m0 = (j % XBLK) * P if j < N_XBLK * XBLK else 0
                pair, half = j // 2, j % 2
                yps = y_ring[pair % 2]
                off = half * 1024
                if do_matmul:
                    for k in range(3):
                        for n0, n1 in ((0, 512), (512, CO)):
                            nc.tensor.matmul(
                                yps[:, off + n0 : off + n1],
                                xb[:, k, m0 : m0 + P],
                                wtb3[:, k, n0:n1],
                                start=(k == 0),
                                stop=(k == 2),
                            )
                last = j == N_TILES - 1
                if not (half == 1 or last):
                    return
                # pair-batched stage 2 (h = 1 for the lone final tile)
                h = 1 if (last and half == 0) else 2
                ypair = y_ring[pair % 2].rearrange("p (s n) -> p s n", n=1024)
                if do_stage2:
                    ysb = ysb_v[pair % 2]
                    nc.scalar.copy(ysb[:, 0:h, :], ypair[:, 0:h, 0:CO])
                    zv = z_v[pair % 2]
                    yv = ysb_ring[pair % 2].rearrange(
                        "p (s o c) -> p s o c", s=2, c=N_CLUSTER
                    )
                    pbc = pb3[:, 2 * pair : 2 * pair + h, :].unsqueeze(2).broadcast_to(
                        [P, h, OUT_DIM, N_CLUSTER]
                    )
                    nc.vector.tensor_tensor(
                        zv[:, 0:h], yv[:, 0:h], pbc, mybir.AluOpType.mult
                    )
                    with nc.allow_low_precision(
                        reason="8-term cluster sum in bf16; error ~0.3% vs 2e-2 gate"
                    ):
                        nc.vector.tensor_reduce(
                            o_v[pair % 2][:, 0:h, :], zv[:, 0:h],
                            mybir.AxisListType.X, mybir.AluOpType.add,
                        )
                if do_store:
                    nc.gpsimd.dma_start(
                        o_blk[:, pair, 0:h, :], o_v[pair % 2][:, 0:h, :]
                    )

            if loop_iters is not None:
                # hardware loop for fast dev timing (per-iter all-engine
                # barrier adds a constant bias; use for relative comparisons)
                with tc.For_i(0, loop_iters):
                    for j in range(N_TILES):
                        tile_body(j)
            else:
                for _ in range(nrep):
                    for j in range(N_TILES):
                        tile_body(j)

    split_multi_waits(nc)
    return nc


def pack_inputs(x, prob, W, b):
    """Host-side packing. Returns per-core input maps."""
    x = np.asarray(x, dtype=np.float32).reshape(TOK, IN_DIM)
    prob = np.asarray(prob, dtype=np.float32).reshape(TOK, N_CLUSTER)
    W = np.asarray(W, dtype=np.float32)
    b = np.asarray(b, dtype=np.float32)

    # weights: wt[i, o*8+c] = W[c,o,i]; bias row at i=336; zeros to IN_P
    wt = np.zeros((IN_P, CO), dtype=np.float32)
    wt[:IN_DIM] = W.transpose(2, 1, 0).reshape(IN_DIM, CO)
    wt[IN_DIM] = b.T.reshape(CO)
    wt16 = np.ascontiguousarray(wt.astype(ml_dtypes.bfloat16))

    in_maps = []
    for c in range(N_CORES):
        xs = x[c * TPC : (c + 1) * TPC]
        # xT [384, 5248] bf16: rows 0:336 = x.T, row 336 = ones, rest zero
        xT = np.zeros((IN_P, TPAD), dtype=ml_dtypes.bfloat16)
        xT[:IN_DIM, :TPC] = xs.T.astype(ml_dtypes.bfloat16)
        xT[IN_DIM, :] = ml_dtypes.bfloat16(1.0)
        xT = np.ascontiguousarray(xT)

        ps = prob[c * TPC : (c + 1) * TPC]
        pp = np.zeros((TPAD, N_CLUSTER), dtype=np.float32)
        pp[:TPC] = ps
        # (j, p, c) -> (p, j, c)
        pp = pp.reshape(N_TILES, P, N_CLUSTER).transpose(1, 0, 2)
        pp16 = np.ascontiguousarray(
            pp.astype(ml_dtypes.bfloat16).reshape(P, N_TILES * N_CLUSTER)
        )
        in_maps.append({"xt": xT, "probp": pp16, "wt": wt16})
    return in_maps


_cached = {}


def kernel(x, prob, W, b):
    key = "main"
    if key not in _cached:
        _cached[key] = build_nc(nrep=1)
    nc = _cached[key]
    in_maps = pack_inputs(x, prob, W, b)
    res = run_bass_kernel_spmd(nc, in_maps, list(range(N_CORES)))
    outs = [res.results[c]["out"][:TPC] for c in range(N_CORES)]
    out = np.concatenate(outs, axis=0).astype(np.float32).reshape(
        BSZ, N_VARS, OUT_DIM
    )
    return out


if __name__ == "__main__":
    rng = np.random.default_rng(0)
    x = rng.standard_normal((BSZ, N_VARS, IN_DIM)).astype(np.float32)
    prob = rng.random((BSZ, N_VARS, N_CLUSTER)).astype(np.float32)
    W = (rng.standard_normal((N_CLUSTER, OUT_DIM, IN_DIM)) / 18.3).astype(np.float32)
    b = rng.standard_normal((N_CLUSTER, OUT_DIM)).astype(np.float32) / 18.3
    out = kernel(x, prob, W, b)
    ref = np.einsum("ti,coi,tc->to", x.reshape(TOK, IN_DIM), W,
                    prob.reshape(TOK, N_CLUSTER)) + prob.reshape(TOK, N_CLUSTER) @ b
    ref = ref.reshape(BSZ, N_VARS, OUT_DIM)
    err = np.linalg.norm(out - ref) / np.linalg.norm(ref)
    print("rel_l2:", err)


# revision 20
# speedup vs baseline: 1.2109x; 1.2109x over previous
"""Cluster-wise linear (MoE-style dense routing) Trainium2 kernel, v2.

Computes out[t,o] = sum_c prob[t,c] * (x[t] @ W[c].T + b[c])[o] for
x (128,321,336) f32, prob (128,321,8), W (8,96,336), b (8,96).

Strategy: data-parallel over 8 NeuronCores (tokens = batch*n_vars split
evenly, 5136/core padded to 41 tiles of 128).

v3 design (vs the v1 baseline's 2688 PE rows/tile):
  - x is transposed + cast to bf16 on the HOST (xT [384, 5248] with a
    ones row at 336 so the bias folds into the matmul). This removes the
    3 PE transposes + DVE copy per tile (384 of 2688 PE rows) and halves
    x HBM traffic. PE per tile = 6 matmuls = 2304 rows ~ 960ns @2.4GHz,
    the roofline engine (41 tiles -> ~39.4us/core steady state).
  - x loads batched 4 tiles per DMA (1024B contiguous lines, avoids the
    <512B descriptor half-rate penalty), issued from sync (HWDGE).
    Output stores batched 2 tiles per DMA on gpsimd (SWDGE).
  - output written bf16 (host casts back to f32): halves out traffic and
    keeps the DVE 2x packing mode on the cluster reduce.
  - stage 2 is PAIR-batched: PSUM = 2 mega-tiles of 2048 f32 (4 banks
    each), a pair of tiles lives at bank-aligned 1024-col slots; one
    Act copy + one DVE mult + one DVE reduce covers 2 tiles, halving
    per-instruction overheads and semaphore traffic.
Per-tile engine budget: PE 6 MM ~960ns (critical), Act copy ~710ns,
DVE mult+reduce ~860ns, DMA engines ~550ns, all hidden behind PE.
"""

import numpy as np
import ml_dtypes

import concourse.bass as bass
import concourse.mybir as mybir
import concourse.tile as tile
from concourse.bass_utils import run_bass_kernel_spmd

N_CORES = 8
BSZ, N_VARS, IN_DIM, OUT_DIM, N_CLUSTER = 128, 321, 336, 96, 8
TOK = BSZ * N_VARS            # 41088
TPC = TOK // N_CORES          # 5136 tokens per core
P = 128
N_TILES = (TPC + P - 1) // P  # 41
TPAD = N_TILES * P            # 5248 padded tokens per core
N_QUAD = (N_TILES + 3) // 4   # 11 output quads (4-tile store batches)
OPAD = N_QUAD * 4 * P         # 5632 padded rows in the out tensor
IN_P = 384                    # padded input dim: 336 data + 1 ones + 47 zeros
CO = OUT_DIM * N_CLUSTER      # 768, o-major: co = o*8 + c
XBLK = 4                      # tiles per x-load DMA
N_XBLK = N_TILES // XBLK      # 10 full blocks
XREM = N_TILES - N_XBLK * XBLK  # 1 leftover tile


def split_multi_waits(nc):
    """This walrus build only supports one sync-wait per instruction; hoist
    extra waits onto same-engine nops inserted immediately before."""
    n_split = 0
    for fn in nc.m.functions:
        for bb in fn.blocks:
            insts = bb.instructions
            out = []
            changed = False
            for inst in insts:
                si = inst.sync_info
                if si is not None and si.on_wait and len(si.on_wait) > 1:
                    waits = list(si.on_wait)
                    del si.on_wait[1:]
                    si.on_wait[0] = waits[-1]
                    for w in waits[:-1]:
                        nop = mybir.InstNoOp(
                            name=f"{inst.name}-wsplit-{n_split}", ins=[], outs=[]
                        )
                        n_split += 1
                        nop.engine = inst.engine
                        nop.sync_info = mybir.SyncInfo(on_wait=[w], on_update=[])
                        out.append(nop)
                        changed = True
                out.append(inst)
            if changed:
                insts[:] = out
    return n_split


def build_nc(nrep: int = 1, bufs: int = 3, loop_iters: int | None = None,
             do_load=True, do_matmul=True, do_stage2=True, do_store=True,
             x_dma_sync=True, out_dma_sync=False):
    nc = bass.Bass()
    dt = mybir.dt
    x_d = nc.dram_tensor("xt", [IN_P, TPAD], dt.bfloat16, kind="ExternalInput")
    p_d = nc.dram_tensor(
        "probp", [P, N_TILES * N_CLUSTER], dt.bfloat16, kind="ExternalInput"
    )
    w_d = nc.dram_tensor("wt", [IN_P, CO], dt.bfloat16, kind="ExternalInput")
    o_d = nc.dram_tensor("out", [OPAD, OUT_DIM], dt.bfloat16, kind="ExternalOutput")

    x_eng = nc.sync if x_dma_sync else nc.gpsimd

    with tile.TileContext(nc) as tc:
        with (
            tc.tile_pool(name="const", bufs=1) as const,
            tc.tile_pool(name="work", bufs=1) as work,
            tc.tile_pool(name="psum", bufs=1, space="PSUM") as psum,
        ):
            # one-time loads
            wtb = const.tile([P, 3 * CO], dt.bfloat16)
            wtb3 = wtb.rearrange("p (k n) -> p k n", k=3)
            nc.gpsimd.dma_start(wtb3[:], w_d.rearrange("(k p) n -> p k n", p=P))
            pball = const.tile([P, N_TILES * N_CLUSTER], dt.bfloat16)
            nc.gpsimd.dma_start(pball[:], p_d[:])
            pb3 = pball.rearrange("p (j c) -> p j c", c=N_CLUSTER)

            # rings. PSUM: 3 single-tile buffers (2 banks each) give the PE
            # 3 tiles of slack before it waits on the Act eviction. Act
            # copies per tile land in adjacent halves of a pair-slot so the
            # DVE mult/reduce still cover 2 tiles per instruction. Output
            # stores batch 4 tiles per DMA to halve Q7 descriptor-gen load.
            xb_ring = [
                work.tile([P, 3 * XBLK * P], dt.bfloat16, name=f"xb{i}")
                for i in range(3)
            ]
            xb_v = [t.rearrange("p (k t) -> p k t", k=3) for t in xb_ring]
            y_ring = [
                psum.tile([P, CO], dt.float32, name=f"yps{i}") for i in range(bufs)
            ]
            ysb_ring = [
                work.tile([P, 2 * CO], dt.bfloat16, name=f"ysb{i}") for i in range(2)
            ]
            ysb_v = [t.rearrange("p (h n) -> p h n", h=2) for t in ysb_ring]
            z_ring = [
                work.tile([P, 2 * CO], dt.bfloat16, name=f"z{i}") for i in range(2)
            ]
            z_v = [t.rearrange("p (h o c) -> p h o c", h=2, c=N_CLUSTER)
                   for t in z_ring]
            o_ring = [
                work.tile([P, 4 * OUT_DIM], dt.bfloat16, name=f"osb{i}")
                for i in range(2)
            ]
            o_v = [t.rearrange("p (h o) -> p h o", h=4) for t in o_ring]

            # DRAM views
            x_full = x_d.rearrange("(k p) t -> p k t", p=P)
            o_blk = o_d.rearrange("(q h p) o -> p q h o", h=4, p=P)

            # the lone final tile stores a half-garbage pair into padded
            # DRAM rows; memset once so the padding lane is finite
            for t in o_ring:
                nc.vector.memset(t[:], 0.0)

            def tile_body(j: int):
                if do_load:
                    if j % XBLK == 0 and j // XBLK < N_XBLK:
                        q = j // XBLK
                        xb = xb_v[q % 3]
                        x_eng.dma_start(
                            xb[:], x_full[:, :, q * XBLK * P : (q + 1) * XBLK * P]
                        )
                    elif j == N_XBLK * XBLK:
                        # leftover tile: load into first part of next ring slot
                        xb = xb_v[(j // XBLK) % 3]
                        x_eng.dma_start(
                            xb[:, :, 0:P], x_full[:, :, j * P : (j + 1) * P]
                        )
                xb = xb_v[(j // XBLK) % 3]
                m0 = (j % XBLK) * P if j < N_XBLK * XBLK else 0
                pair, half = j // 2, j % 2
                yps = y_ring[j % bufs]
                if do_matmul:
                    for k in range(3):
                        for n0, n1 in ((0, 512), (512, CO)):
                            nc.tensor.matmul(
                                yps[:, n0:n1],
                                xb[:, k, m0 : m0 + P],
                                wtb3[:, k, n0:n1],
                                start=(k == 0),
                                stop=(k == 2),
                            )
                last = j == N_TILES - 1
                if do_stage2:
                    # per-tile PSUM eviction into the pair slot (frees the
                    # PSUM buffer as early as possible for the PE)
                    nc.scalar.copy(ysb_v[pair % 2][:, half, :], yps[:])
                if not (half == 1 or last):
                    return
                # pair-batched mult+reduce (h = 1 for the lone final tile)
                h = 1 if (last and half == 0) else 2
                quad, qh = pair // 2, 2 * (pair % 2)
                if do_stage2:
                    zv = z_v[pair % 2]
                    yv = ysb_ring[pair % 2].rearrange(
                        "p (s o c) -> p s o c", s=2, c=N_CLUSTER
                    )
                    pbc = pb3[:, 2 * pair : 2 * pair + h, :].unsqueeze(2).broadcast_to(
                        [P, h, OUT_DIM, N_CLUSTER]
                    )
                    nc.vector.tensor_tensor(
                        zv[:, 0:h], yv[:, 0:h], pbc, mybir.AluOpType.mult
                    )
                    with nc.allow_low_precision(
                        reason="8-term cluster sum in bf16; error ~0.3% vs 2e-2 gate"
                    ):
                        nc.vector.tensor_reduce(
                            o_v[quad % 2][:, qh : qh + h, :], zv[:, 0:h],
                            mybir.AxisListType.X, mybir.AluOpType.add,
                        )
                if do_store and (pair % 2 == 1 or last):
                    nc.gpsimd.dma_start(
                        o_blk[:, quad, 0 : qh + h, :], o_v[quad % 2][:, 0 : qh + h, :]
                    )

            if loop_iters is not None:
                # hardware loop for fast dev timing (per-iter all-engine
                # barrier adds a constant bias; use for relative comparisons)
                with tc.For_i(0, loop_iters):
                    for j in range(N_TILES):
                        tile_body(j)
            else:
                for _ in range(nrep):
                    for j in range(N_TILES):
                        tile_body(j)

    split_multi_waits(nc)
    return nc


def pack_inputs(x, prob, W, b):
    """Host-side packing. Returns per-core input maps."""
    x = np.asarray(x, dtype=np.float32).reshape(TOK, IN_DIM)
    prob = np.asarray(prob, dtype=np.float32).reshape(TOK, N_CLUSTER)
    W = np.asarray(W, dtype=np.float32)
    b = np.asarray(b, dtype=np.float32)

    # weights: wt[i, o*8+c] = W[c,o,i]; bias row at i=336; zeros to IN_P
    wt = np.zeros((IN_P, CO), dtype=np.float32)
    wt[:IN_DIM] = W.transpose(2, 1, 0).reshape(IN_DIM, CO)
    wt[IN_DIM] = b.T.reshape(CO)
    wt16 = np.ascontiguousarray(wt.astype(ml_dtypes.bfloat16))

    in_maps = []
    for c in range(N_CORES):
        xs = x[c * TPC : (c + 1) * TPC]
        # xT [384, 5248] bf16: rows 0:336 = x.T, row 336 = ones, rest zero
        xT = np.zeros((IN_P, TPAD), dtype=ml_dtypes.bfloat16)
        xT[:IN_DIM, :TPC] = xs.T.astype(ml_dtypes.bfloat16)
        xT[IN_DIM, :] = ml_dtypes.bfloat16(1.0)
        xT = np.ascontiguousarray(xT)

        ps = prob[c * TPC : (c + 1) * TPC]
        pp = np.zeros((TPAD, N_CLUSTER), dtype=np.float32)
        pp[:TPC] = ps
        # (j, p, c) -> (p, j, c)
        pp = pp.reshape(N_TILES, P, N_CLUSTER).transpose(1, 0, 2)
        pp16 = np.ascontiguousarray(
            pp.astype(ml_dtypes.bfloat16).reshape(P, N_TILES * N_CLUSTER)
        )
        in_maps.append({"xt": xT, "probp": pp16, "wt": wt16})
    return in_maps


_cached = {}


def kernel(x, prob, W, b):
    key = "main"
    if key not in _cached:
        _cached[key] = build_nc(nrep=1)
    nc = _cached[key]
    in_maps = pack_inputs(x, prob, W, b)
    res = run_bass_kernel_spmd(nc, in_maps, list(range(N_CORES)))
    outs = [res.results[c]["out"][:TPC] for c in range(N_CORES)]
    out = np.concatenate(outs, axis=0).astype(np.float32).reshape(
        BSZ, N_VARS, OUT_DIM
    )
    return out


if __name__ == "__main__":
    rng = np.random.default_rng(0)
    x = rng.standard_normal((BSZ, N_VARS, IN_DIM)).astype(np.float32)
    prob = rng.random((BSZ, N_VARS, N_CLUSTER)).astype(np.float32)
    W = (rng.standard_normal((N_CLUSTER, OUT_DIM, IN_DIM)) / 18.3).astype(np.float32)
    b = rng.standard_normal((N_CLUSTER, OUT_DIM)).astype(np.float32) / 18.3
    out = kernel(x, prob, W, b)
    ref = np.einsum("ti,coi,tc->to", x.reshape(TOK, IN_DIM), W,
                    prob.reshape(TOK, N_CLUSTER)) + prob.reshape(TOK, N_CLUSTER) @ b
    ref = ref.reshape(BSZ, N_VARS, OUT_DIM)
    err = np.linalg.norm(out - ref) / np.linalg.norm(ref)
    print("rel_l2:", err)
